# revision 50
# baseline (speedup 1.0000x reference)
"""Trainium2 Bass kernel for the fused einsum/groupconv/bmm module.

Math (per image n, C=256, H=W=56, HW=3136):
  t1[c,e] = sum_s X[c,s] P[s,e]
  t3      = groupconv3x3(x[n], conv_w, groups=2)
  t4      = p4 * t1;  t5[a] = sum_b t4[a,b] p5[b]
  out     = (t4 @ t3)/16 + broadcast((t5 @ X)/16)

Design (8 cores, 4 images each; HW-measured ~191us/core warm (~229us
when the chip sits in the P0 power-state downclock, 2.4->2.0GHz); PE
~100% busy in steady state, MATMUL busy ~159us):
  - Everything HBM-resident is bf16 (tolerance 2e-2 >> measured 4.4e-3);
    output bf16 on device, converted to f32 on host.  fp8 was tried for
    the conv (DoubleRow tap-pairing, 1.6x faster) but fails accuracy:
    for random-sign dot products the relative error EQUALS the e4m3
    element quantization error (~3.6% RMS), it does not average down.
  - X^T for t1 via ONE batched DMA-transpose per image straight from
    DRAM (k=0..23; the w=64 tail chunk via a PE transpose reading a
    small dedicated xtail load) -- keeps ~8us/image of transposes +
    LDWEIGHTS off the PE, which is the bottleneck engine.  All
    transposes are issued on the SYNC HWDGE ring, strictly one per
    image between that image's own loads: a DMA-transpose on one ring
    concurrent with SBUF-writing copies on the other corrupts data
    (shared S2M xbar mode), and batching several images' transposes
    back-to-back produced transient NaNs.
  - The padded 58x58 conv layout is built ON-CHIP: persistent pad-zeroed
    tiles, interior rewritten per image by DVE (bf16 SBUF copies run 4x
    there; GpSimd measured 10.6us per copy on HW and serialized the
    whole pipeline -- the cost model underprices GpSimd by ~10x).
  - conv: 9 shifted matmuls per group into PSUM, rhs as a 2-free-dim AP
    [8 rows x 56 cols, row stride 58] so only the 448 real output
    columns are streamed; the whole chunk pipeline is 448-compact.
  - t7: per chunk, one broadcast PSUM tile via 2 matmuls with a
    stationary t5-broadcast matrix (t5b[b,a] = t5[b] for all a), bounced
    to SBUF once on ACT; the two PSUM->output drains are DVE adds.  The
    t7 rhs reads the PADDED tiles so xraw releases right after the
    pad-build and the next image's x load starts ~2 images early.
  - Output staged per 2-chunk tiles, stored as soon as complete.
  - Image 0 runs conv-g0 first (needs only x-cb0+wt, the earliest DMAs);
    the sync ring orders cb0 / wt / cb0-tail / transposes / cb1 to match
    consumption order.
"""

import sys

sys.path.insert(0, "/opt/trn_rl_repo")

import numpy as np

N, C, H, W = 32, 256, 56, 56
HW = H * W            # 3136
PH = H + 2            # 58
PHW = PH * PH         # 3364
XLEN = PHW + 3        # guard elem each end + 1 for the compact-conv AP view
NCORES = 8
NPER = N // NCORES    # 4 images per core
CHP = 8 * PH          # padded chunk: 8 padded rows = 464
NCHUNK = 7            # row starts 1,9,...,49 cover out rows 1..56
KP = 128              # transpose chunk (contiguous in UNPADDED x)
KT = 25               # ceil(HW/KP); t1 contraction over unpadded s
KDMA = 24             # k-chunks transposed by the DMA xbar (full 128 wide)
SLAST = HW - 24 * KP  # 64: width of the last (partial) transpose chunk
INV = 1.0 / 16.0      # 1/sqrt(C)


def build_body(tc, outs, ins):
    import concourse.mybir as mybir

    nc = tc.nc
    f32 = mybir.dt.float32
    bf16 = mybir.dt.bfloat16

    x_d = ins["x"]          # (NPER, C, HW)      bf16
    p1_d = ins["p1"]        # (KP, KT*C)         bf16 (pad rows zero)
    wt_d = ins["wt"]        # (128, 2*9*128)     bf16 [i, (g,t,o)]
    p4_d = ins["p4s"]       # (128, 2*C)         f32  [b, (bb,a)], pre-scaled
    p5_d = ins["p5"]        # (128, 2)           bf16 [b, bb]
    out_d = outs["out"]     # (NPER, C, HW)      bf16

    with (
        tc.tile_pool(name="const", bufs=1) as constp,
        tc.tile_pool(name="xrawp", bufs=2) as xrawp,
        tc.tile_pool(name="xtp", bufs=2) as xtp,
        tc.tile_pool(name="t3p", bufs=4) as t3p,
        tc.tile_pool(name="t3p0", bufs=7) as t3p0,
        tc.tile_pool(name="svp", bufs=2) as svp,
        tc.tile_pool(name="outp", bufs=3) as outp,
        tc.tile_pool(name="ps_tail", bufs=1, space="PSUM") as ps_tail,
        tc.tile_pool(name="ps_acc", bufs=1, space="PSUM") as ps_acc,
        tc.tile_pool(name="ps_cv", bufs=3, space="PSUM") as ps_cv,
        tc.tile_pool(name="ps_t6", bufs=2, space="PSUM") as ps_t6,
        tc.tile_pool(name="ps_t7", bufs=1, space="PSUM") as ps_t7,
    ):
        # ---- startup DMAs.  The sync (SP) HWDGE ring carries the big x
        # loads + all DMA-transposes (sync is otherwise idle, and the
        # xbar transpose BLOCKS its issuing sequencer ~3.6us each); the
        # scalar (ACT) ring carries constants + the per-image stores
        # (whose dispatch is data-dependent and must not delay loads).
        # Image-0 ordering: x cb0 lands first, conv g0 runs on it while
        # cb1 / xt / t1 catch up. ----
        xraw0 = xrawp.tile([128, 2 * HW], bf16, tag="xraw", name="xraw")
        nc.sync.dma_start(out=xraw0[:, 0 : 10 * W], in_=x_d[0, 0:128, 0 : 10 * W])
        ident = constp.tile([128, 128], bf16, name="ident")
        nc.scalar.dma_start(out=ident[:, :], in_=ins["ident"])
        p4_sb = constp.tile([128, 2 * C], f32, name="p4_sb")
        nc.scalar.dma_start(out=p4_sb[:, :], in_=p4_d)
        p5_sb = constp.tile([128, 2], bf16, name="p5_sb")
        nc.scalar.dma_start(out=p5_sb[:, :], in_=p5_d)
        ones_sb = constp.tile([128, 128], bf16, name="ones_sb")
        nc.vector.memset(ones_sb[:, :], 1.0)

        p1_sb = constp.tile([KP, KT * C], bf16, name="p1_sb")
        nc.scalar.dma_start(
            out=p1_sb[:, 0 : 13 * C], in_=p1_d[:, 0 : 13 * C]
        )
        wt_sb = constp.tile([128, 2 * 9 * 128], bf16, name="wt_sb")
        nc.sync.dma_start(out=wt_sb[:, :], in_=wt_d)
        nc.scalar.dma_start(
            out=p1_sb[:, 13 * C : KT * C], in_=p1_d[:, 13 * C : KT * C]
        )

        # persistent padded-x tiles: pad positions zeroed once (head, the
        # 2-wide row seams, tail); per image only interior cols are
        # rewritten, pads stay zero.
        xpads = [
            [
                constp.tile([128, XLEN], bf16, name=f"xpad{q}{cb}")
                for cb in range(2)
            ]
            for q in range(2)
        ]
        for q in range(2):
            for cb in range(2):
                xp = xpads[q][cb]
                eng = nc.vector if q == 0 else nc.gpsimd
                eng.memset(xp[:, 0:60], 0.0)
                eng.memset(
                    xp[:, 116 : 116 + 55 * PH].rearrange(
                        "p (r w) -> p r w", w=PH
                    )[:, :, 0:2],
                    0.0,
                )
                eng.memset(xp[:, 3306:XLEN], 0.0)

        for n in range(NPER):
            q = n % 2
            xp0, xp1 = xpads[q]

            # ---- X^T via one batched xbar transpose per image (24
            # k-chunks; w=64 tail via PE).  ALL transposes ride the sync
            # ring: a DMA-transpose issued on one HWDGE ring concurrent
            # with SBUF-writing copy DMAs on the other corrupts data
            # (shared S2M xbar mode -- known HW bug; same-ring issue is
            # serialized, cross-ring is NOT).  Batching transposes of
            # several images back-to-back against in-flight copies also
            # produced transient NaNs -- keep strictly one per image,
            # sandwiched between that image's own loads. ----
            xt = xtp.tile([KP, KT * C], bf16, tag="xt", name="xt")
            # small dedicated load of x's tail columns (s=3072..3136) so
            # the PE tail-transpose does not depend on the big cb1 load
            # (PE executes in program order -- a waiting tail-transpose
            # would block t1's ready matmuls behind it)
            xtail = xtp.tile([128, 2 * SLAST], bf16, tag="xtail", name="xtail")

            def xt_transpose():
                nc.sync.dma_start(
                    out=xt[:, 0 : KDMA * C].rearrange(
                        "p (k c) -> p k c", c=C
                    ),
                    in_=x_d[n, :, 0 : KDMA * KP],
                    transpose=True,
                )

            if n == 0:
                # cb0 lands in 3 pieces sized to conv-chunk consumption
                # (rows 0-9 -> chunk 0, 10-33 -> chunks 1-3, 34-55 -> 4-6)
                # with wt riding after the first, so conv g0 starts ~11us
                # in and never waits on the rest.  The transpose sits
                # between the cb0 tail and cb1 so conv g0 / t1 / conv g1
                # become ready in consumption order.
                xraw = xraw0
                nc.sync.dma_start(
                    out=xraw[:, 10 * W : 34 * W], in_=x_d[0, 0:128, 10 * W : 34 * W]
                )
                nc.sync.dma_start(
                    out=xraw[:, 34 * W : HW], in_=x_d[0, 0:128, 34 * W : HW]
                )
                nc.sync.dma_start(
                    out=xtail.rearrange("p (cb s) -> p cb s", cb=2),
                    in_=x_d[n].rearrange("(cb p) s -> p cb s", cb=2)[
                        :, :, KDMA * KP : HW
                    ],
                )
                xt_transpose()
                nc.sync.dma_start(
                    out=xraw[:, HW : 2 * HW], in_=x_d[0, 128:256, :]
                )
            else:
                xraw = xrawp.tile([128, 2 * HW], bf16, tag="xraw", name="xraw")
                nc.sync.dma_start(out=xraw[:, 0:HW], in_=x_d[n, 0:128, :])
                nc.sync.dma_start(
                    out=xtail.rearrange("p (cb s) -> p cb s", cb=2),
                    in_=x_d[n].rearrange("(cb p) s -> p cb s", cb=2)[
                        :, :, KDMA * KP : HW
                    ],
                )
                xt_transpose()
                nc.sync.dma_start(
                    out=xraw[:, HW : 2 * HW], in_=x_d[n, 128:256, :]
                )

            # ---- tail transpose chunk (w=64) on the PE, from xtail ----
            trp = ps_tail.tile([KP, 256], bf16, tag="tr", name="trp")
            for cb in range(2):
                nc.tensor.transpose(
                    trp[0:SLAST, cb * 128 : cb * 128 + 128],
                    xtail[:, cb * SLAST : (cb + 1) * SLAST],
                    ident[:, :],
                )
            nc.vector.tensor_copy(
                xt[0:SLAST, KDMA * C : KT * C], trp[0:SLAST, :]
            )

            # ---- build padded interior on-chip, on DVE (bf16 SBUF->SBUF
            # copies run 4x there; HW GpSimd took ~10.6us each).  Image-0
            # cb0 is built in two row-halves so conv g0 chunk 0 starts as
            # soon as the first half of the x load lands. ----
            for cb, xp in ((0, xp0), (1, xp1)):
                halves = (
                    ((0, 10), (10, 34), (34, 56))
                    if (n == 0 and cb == 0)
                    else ((0, 56),)
                )
                for r0h, r1h in halves:
                    dst = xp[
                        :, 60 + r0h * PH : 60 + r1h * PH
                    ].rearrange("p (r w) -> p r w", w=PH)[:, :, 0:56]
                    src = xraw[
                        :, cb * HW + r0h * W : cb * HW + r1h * W
                    ].rearrange("p (r w) -> p r w", w=W)
                    nc.vector.tensor_copy(dst, src)

            def conv_g(g, xp, c, pool):
                r0 = 1 + 8 * c
                # stream only the 448 real output columns per tap: the rhs
                # is a 2-free-dim AP [8 rows x 56 cols] with row stride 58,
                # skipping the 16 dead pad columns of the 464-wide window
                cv = ps_cv.tile([128, 448], f32, tag="cv", name="cv")
                for tap in range(9):
                    kh, kw = tap // 3, tap % 3
                    foff = (r0 + kh - 1) * PH + kw
                    nc.tensor.matmul(
                        cv[:, :],
                        wt_sb[
                            :, (g * 9 + tap) * 128 : (g * 9 + tap) * 128 + 128
                        ],
                        xp[:, foff + 1 : foff + 1 + 8 * PH].rearrange(
                            "p (r w) -> p r w", w=PH
                        )[:, :, 0:56],
                        start=(tap == 0),
                        stop=(tap == 8),
                    )
                t3g = pool.tile([128, 448], bf16, tag=f"t3{g}", name="t3g")
                nc.scalar.copy(t3g[:, :], cv[:, :])
                return t3g

            # ---- image 0: run ALL of conv g0 first (needs only x-cb0 +
            # wt, the earliest-landing DMAs) to cover the startup DMA
            # window; t1 waits on xt/p1 which land later ----
            t3g0s = []
            if n == 0:
                for c in range(NCHUNK):
                    t3g0s.append(conv_g(0, xp0, c, t3p0))

            # ---- t1T = P^T @ X^T;  t4T = p4s * t1T  (b-part, a-free) ----
            t4T = svp.tile([128, 2 * C], bf16, tag="t4", name="t4T")
            for eb in range(2):
                t1ps = ps_acc.tile([128, C], f32, tag="acc", name="t1ps")
                for k in range(KT):
                    w = KP if k < KT - 1 else SLAST
                    nc.tensor.matmul(
                        t1ps[:, :],
                        p1_sb[0:w, k * C + eb * 128 : k * C + eb * 128 + 128],
                        xt[0:w, k * C : (k + 1) * C],
                        start=(k == 0),
                        stop=(k == KT - 1),
                    )
                nc.vector.tensor_mul(
                    t4T[:, eb * C : (eb + 1) * C],
                    t1ps[:, :],
                    p4_sb[:, eb * C : (eb + 1) * C],
                )

            # ---- t5[a] = sum_b t4T[b,a] p5[b] ----
            t5ps = ps_acc.tile([128, 2], f32, tag="acc", name="t5ps")
            for ab in range(2):
                for bb in range(2):
                    nc.tensor.matmul(
                        t5ps[:, ab : ab + 1],
                        t4T[:, bb * C + ab * 128 : bb * C + ab * 128 + 128],
                        p5_sb[:, bb : bb + 1],
                        start=(bb == 0),
                        stop=(bb == 1),
                    )
            t5col = svp.tile([128, 2], f32, tag="t5", name="t5col")
            nc.scalar.copy(t5col[:, :], t5ps[:, :])
            # t5 broadcast as a stationary matrix: t5b[cb][b, a] = t5[cb*128+b]
            # for every a -- the t7 row-broadcast then rides the PE.
            t5b = svp.tile([128, 2 * 128], bf16, tag="t5b", name="t5b")
            for cb in range(2):
                nc.vector.tensor_scalar_mul(
                    t5b[:, cb * 128 : (cb + 1) * 128],
                    ones_sb[:, :],
                    t5col[:, cb : cb + 1],
                )

            # ---- chunk loop: conv then t6(+t7).  Output staged per
            # 2-chunk tiles (stored as soon as complete) so SBUF holds
            # ~0.5MB of output instead of two full images. ----
            ob = None
            for c in range(NCHUNK):
                if c % 2 == 0:
                    obw = 2 * 448 if c < 6 else 448
                    ob = outp.tile([128, 2 * obw], bf16, tag="ob", name="ob")
                if n == 0:
                    t3c = [t3g0s[c], conv_g(1, xp1, c, t3p)]
                else:
                    t3c = [conv_g(0, xp0, c, t3p), conv_g(1, xp1, c, t3p)]

                # broadcast t7 row for this chunk: every output partition
                # of t7ps gets t7[s] (t5b columns are all equal).  The rhs
                # reads the PADDED tiles (center-tap window, same AP shape
                # as the conv) so xraw is released right after the
                # pad-build and the next x load can start 2 images early.
                r0c = 1 + 8 * c
                t7ps = ps_t7.tile([128, 448], f32, tag="t7", name="t7ps")
                for cb, xp in ((0, xp0), (1, xp1)):
                    nc.tensor.matmul(
                        t7ps[:, :],
                        t5b[:, cb * 128 : (cb + 1) * 128],
                        xp[:, r0c * PH + 2 : r0c * PH + 2 + 8 * PH].rearrange(
                            "p (r w) -> p r w", w=PH
                        )[:, :, 0:56],
                        start=(cb == 0),
                        stop=(cb == 1),
                    )
                t7b = svp.tile([128, 448], f32, tag="t7b", name="t7b")
                nc.scalar.copy(t7b[:, :], t7ps[:, :])

                co = (c % 2) * 448
                for ab in range(2):
                    t6ps = ps_t6.tile([128, 448], f32, tag="t6", name="t6ps")
                    for bb in range(2):
                        nc.tensor.matmul(
                            t6ps[:, :],
                            t4T[:, bb * C + ab * 128 : bb * C + ab * 128 + 128],
                            t3c[bb][:, :],
                            start=(bb == 0),
                            stop=(bb == 1),
                        )
                    nc.vector.tensor_add(
                        ob[:, ab * obw + co : ab * obw + co + 448],
                        t6ps[:, :],
                        t7b[:, :],
                    )

                if c % 2 == 1 or c == 6:
                    lo = (c // 2) * 2 * 448
                    nc.scalar.dma_start(
                        out=out_d[n, :, lo : lo + obw].rearrange(
                            "(ab p) s -> p ab s", ab=2
                        ),
                        in_=ob.rearrange("p (ab s) -> p ab s", ab=2),
                    )


_CACHE = {}


def _get_nc():
    if "nc" in _CACHE:
        return _CACHE["nc"]
    import concourse.bacc as bacc
    import concourse.mybir as mybir
    import concourse.tile as tile

    f32 = mybir.dt.float32
    bf16 = mybir.dt.bfloat16
    nc = bacc.Bacc(
        "TRN2", target_bir_lowering=False, debug=False, num_devices=NCORES
    )
    ins = {
        "x": nc.dram_tensor("x", (NPER, C, HW), bf16, kind="ExternalInput").ap(),
        "p1": nc.dram_tensor("p1", (KP, KT * C), bf16, kind="ExternalInput").ap(),
        "wt": nc.dram_tensor("wt", (128, 2 * 9 * 128), bf16, kind="ExternalInput").ap(),
        "p4s": nc.dram_tensor("p4s", (128, 2 * C), f32, kind="ExternalInput").ap(),
        "p5": nc.dram_tensor("p5", (128, 2), bf16, kind="ExternalInput").ap(),
        "ident": nc.dram_tensor("ident", (128, 128), bf16, kind="ExternalInput").ap(),
    }
    outs = {
        "out": nc.dram_tensor("out", (NPER, C, HW), bf16, kind="ExternalOutput").ap(),
    }
    with tile.TileContext(nc) as tc:
        build_body(tc, outs, ins)
    nc.compile()
    _CACHE["nc"] = nc
    return nc


def host_prep(inputs):
    """Split full inputs into per-core in_maps (host-side relayout + bf16)."""
    import ml_dtypes

    bf = ml_dtypes.bfloat16
    x = np.asarray(inputs["x"], dtype=np.float32).reshape(N, C, HW).astype(bf)
    p1p = np.zeros((KT * KP, C), dtype=np.float32)
    p1p[:HW] = np.asarray(inputs["p1_w"], dtype=np.float32)[..., 0].reshape(
        HW, C
    )
    p1h = np.ascontiguousarray(
        p1p.reshape(KT, KP, C).transpose(1, 0, 2).reshape(KP, KT * C)
    ).astype(bf)
    wt = np.ascontiguousarray(
        np.asarray(inputs["conv_w"], dtype=np.float32)
        .reshape(2, 128, 128, 9)
        .transpose(3, 0, 1, 2)       # t, g, o, i -> want [i, (g,t,o)]
        .transpose(3, 1, 0, 2)       # i, g, t, o
        .reshape(128, 2 * 9 * 128)
    ).astype(bf)
    p4s = np.ascontiguousarray(
        (np.asarray(inputs["p4_w"], dtype=np.float32)[0].T * INV)
        .reshape(2, 128, C)
        .transpose(1, 0, 2)
        .reshape(128, 2 * C)
    )
    identm = np.eye(128, dtype=np.float32).astype(bf)
    p5 = np.ascontiguousarray(
        np.asarray(inputs["p5_w"], dtype=np.float32).reshape(2, 128).T
    ).astype(bf)
    xs = x.reshape(NCORES, NPER, C, HW)
    return [
        {
            "x": np.ascontiguousarray(xs[i]), "p1": p1h, "wt": wt,
            "p4s": p4s, "p5": p5, "ident": identm,
        }
        for i in range(NCORES)
    ]


def kernel(**inputs):
    from concourse.bass_utils import run_bass_kernel_spmd

    nc = _get_nc()
    in_maps = host_prep(inputs)
    res = run_bass_kernel_spmd(nc, in_maps, core_ids=list(range(NCORES)))
    out = np.concatenate([res.results[i]["out"] for i in range(NCORES)], axis=0)
    return out.astype(np.float32).reshape(N, C, H, W)
